# revision 25
# baseline (speedup 1.0000x reference)
"""Trainium2 Bass kernel for nn_AugmentShallow (gnn_message_passing).

Strength-reduced formulation (see kernel.py docstring): per-point MLP
    p[j] = relu(Weff @ x[j] + beff),  q[j] = relu(Wc1 @ p[j] + bc1)
then m[n] = sum_k q[knn[n,k]] and out = m @ (W2/K).T + b2.

The neighbor-sum is split across two engines that run concurrently:
  * chunks 0..11 (tokens 0..6143): SWDGE dma_gather of q rows from DRAM
    (non-transpose, 4 queues) + PE identity-matmul / DVE strided-reduce
    K-sum.  Q7 descriptor generation (~2.35ns/idx, serial) is the
    critical path of this half.
  * chunks 12..15 (tokens 6144..8191): PE one-hot matmul
    m_cm = sum_jb q_jb.T @ S[jb]  where S is the host-built selection
    matrix ([8192, 2048] fp8, S[j,t] = multiplicity of j among token
    t's neighbors).  The 64 jb-block matmuls are interleaved with the
    gather-chunk emission so they fill the PE while the Pool engine
    generates descriptors.  Output is channel-major, so trans2 consumes
    it directly (no transposes on this half).

Sharding: data-parallel over B -- core i owns batch i.
"""

import sys

if "/opt/trn_rl_repo" not in sys.path:
    sys.path.insert(0, "/opt/trn_rl_repo")

import numpy as np
import ml_dtypes

B, N, K = 8, 8192, 12
C_IN, C_HID, C_OUT = 3, 128, 256

CHUNK = 512                     # output tokens per gather chunk
N_CHUNKS = N // CHUNK           # 16
H = 3                           # chunks handled by the PE one-hot path
NG = N_CHUNKS - H               # gather chunks
PE_T = H * CHUNK                # one-hot tokens (tail of the batch)
IDX_PER_CHUNK = CHUNK * K       # 6144
IDX_SLOTS = IDX_PER_CHUNK // 16  # 384 (16-partition wrap)
G_SLOTS = IDX_PER_CHUNK // 128  # 48
P_TOK = 512                     # tokens per p/q-stage matmul
N_QUEUES = 4
JB = N // 128                   # 64 one-hot contraction blocks

_CACHE = {}


def _build_program():
    import concourse.bacc as bacc
    import concourse.mybir as mybir
    import concourse.tile as tile

    dt = mybir.dt
    nc = bacc.Bacc("TRN2", target_bir_lowering=False, debug=False, num_devices=8,
                   num_swdge_queues=N_QUEUES)

    xT_d = nc.dram_tensor("xT", [C_IN, N], dt.float16, kind="ExternalInput")
    idx_d = nc.dram_tensor("idx", [128, NG * IDX_SLOTS], dt.int16,
                           kind="ExternalInput")
    weffT_d = nc.dram_tensor("weffT", [C_IN, C_HID], dt.float16,
                             kind="ExternalInput")
    beff_d = nc.dram_tensor("beff", [C_HID, 1], dt.float32, kind="ExternalInput")
    wc1T_d = nc.dram_tensor("wc1T", [C_HID, C_HID], dt.float16,
                            kind="ExternalInput")
    bc1_d = nc.dram_tensor("bc1c", [C_HID, 1], dt.float32, kind="ExternalInput")
    w2T_d = nc.dram_tensor("w2T", [C_HID, C_OUT], dt.float16,
                           kind="ExternalInput")
    b2_d = nc.dram_tensor("b2b", [128, C_OUT], dt.float32, kind="ExternalInput")
    ident_d = nc.dram_tensor("ident", [128, 128], dt.float16,
                             kind="ExternalInput")
    s_d = nc.dram_tensor("sel", [N, PE_T], dt.float8e4, kind="ExternalInput")
    out_d = nc.dram_tensor("out", [N, C_OUT], dt.float32, kind="ExternalOutput")

    with tile.TileContext(nc) as tc:
        with (
            tc.tile_pool(name="const", bufs=1) as cpool,
            tc.tile_pool(name="qstage", bufs=2) as qpool,
            tc.tile_pool(name="spool", bufs=3) as spool,
            tc.tile_pool(name="gpool", bufs=9) as gpool,
            tc.tile_pool(name="mpool", bufs=4) as mpool,
            tc.tile_pool(name="mtpool", bufs=3) as mtpool,
            tc.tile_pool(name="opool", bufs=2) as opool,
            tc.tile_pool(name="qdram", bufs=1, space="DRAM") as dpool,
            tc.tile_pool(name="pp", bufs=2, space="PSUM") as pp,
            tc.tile_pool(name="oh", bufs=1, space="PSUM") as ohpool,
            tc.tile_pool(name="pt", bufs=2, space="PSUM") as pt,
            tc.tile_pool(name="po", bufs=1, space="PSUM") as po,
        ):
            # ---- persistent SBUF tensors -------------------------------
            xT = cpool.tile([C_IN, N], dt.float16)
            idx = cpool.tile([128, NG * IDX_SLOTS], dt.int16)
            weffT = cpool.tile([C_IN, C_HID], dt.float16)
            beff = cpool.tile([C_HID, 1], dt.float32)
            wc1T = cpool.tile([C_HID, C_HID], dt.float16)
            bc1 = cpool.tile([C_HID, 1], dt.float32)
            w2T = cpool.tile([C_HID, C_OUT], dt.float16)
            b2 = cpool.tile([128, C_OUT], dt.float32)
            ident = cpool.tile([128, 128], dt.float16)
            p_f16 = cpool.tile([128, N], dt.float16)   # [ch, tok]
            q_sbuf = cpool.tile([128, N], dt.float16)  # token-major stripes
            q_dram = dpool.tile([N, C_HID], dt.float16)  # token-major rows

            nc.sync.dma_start(idx[:], idx_d.ap()[:])
            nc.sync.dma_start(xT[:], xT_d.ap()[:])
            nc.sync.dma_start(weffT[:], weffT_d.ap()[:])
            nc.sync.dma_start(beff[:], beff_d.ap()[:])
            nc.sync.dma_start(wc1T[:], wc1T_d.ap()[:])
            nc.sync.dma_start(bc1[:], bc1_d.ap()[:])
            nc.sync.dma_start(w2T[:], w2T_d.ap()[:])
            nc.sync.dma_start(b2[:], b2_d.ap()[:])
            nc.sync.dma_start(ident[:], ident_d.ap()[:])

            # ---- p = relu(Weff @ x + beff), channel-major [128, N] -----
            for c in range(N // P_TOK):
                ppt = pp.tile([128, P_TOK], dt.float32, tag="ps512")
                nc.tensor.matmul(
                    ppt[:], weffT[:], xT[:, c * P_TOK:(c + 1) * P_TOK],
                    start=True, stop=True,
                )
                nc.scalar.activation(
                    p_f16[:, c * P_TOK:(c + 1) * P_TOK], ppt[:],
                    mybir.ActivationFunctionType.Relu, bias=beff[:],
                )

            # ---- q = relu(Wc1 @ p + bc1); PE-transpose to token-major
            # stripes in q_sbuf (token j at partition j%128, stripe j//128),
            # which both the DRAM staging DMA and the one-hot lhsT use.
            for g4 in range(N // P_TOK):
                qps = pp.tile([128, P_TOK], dt.float32, tag="ps512")
                nc.tensor.matmul(
                    qps[:], wc1T[:], p_f16[:, g4 * P_TOK:(g4 + 1) * P_TOK],
                    start=True, stop=True,
                )
                q_cm = qpool.tile([128, P_TOK], dt.float16, tag="qcm")
                nc.scalar.activation(
                    q_cm[:], qps[:],
                    mybir.ActivationFunctionType.Relu, bias=bc1[:],
                )
                for s in range(P_TOK // 128):
                    tb = g4 * (P_TOK // 128) + s
                    tq = pt.tile([128, 128], dt.float16, tag="tps")
                    nc.tensor.transpose(
                        tq[:], q_cm[:, s * 128:(s + 1) * 128], ident[:])
                    # DVE copy: the head's ACT chain (p/q relus) is the
                    # longer pole; DVE is idle here.
                    nc.vector.tensor_copy(
                        q_sbuf[:, tb * 128:(tb + 1) * 128], tq[:])
                nc.sync.dma_start(
                    q_dram[g4 * P_TOK:(g4 + 1) * P_TOK, :]
                    .rearrange("(s p) o -> p s o", p=128),
                    q_sbuf[:, g4 * P_TOK:(g4 + 1) * P_TOK]
                    .rearrange("p (s o) -> p s o", o=C_HID),
                )

            # one-hot accumulators, live through the whole gather phase
            oh = [ohpool.tile([128, CHUNK], dt.float32, tag=f"oh{h}",
                              name=f"oh{h}")
                  for h in range(H)]
            # spread the 64 jb one-hot steps across gather iterations
            # 1..NG-2: iteration 0 keeps the head's DMA bandwidth for the
            # q_dram staging, and finishing early lets the one-hot chunk
            # tails overlap the last gather chunk.
            NSL = NG - 2
            jb_sched = [[] for _ in range(NG)]
            for i in range(NSL):
                jb_sched[i + 1] = list(range((i * JB) // NSL,
                                             ((i + 1) * JB) // NSL))

            # ---- gather chunks + interleaved one-hot -------------------
            for c in range(NG):
                g = gpool.tile([128, G_SLOTS, 128], dt.float16)
                nc.gpsimd.dma_gather(
                    g[:],
                    q_dram[:],
                    idx[:, c * IDX_SLOTS:(c + 1) * IDX_SLOTS],
                    num_idxs=IDX_PER_CHUNK,
                    num_idxs_reg=IDX_PER_CHUNK,
                    elem_size=C_HID,
                    transpose=False,
                    single_packet=False,
                    queue_num=c % N_QUEUES,
                )
                # one-hot slice: fills the PE while Q7 generates descs
                for jb in jb_sched[c]:
                    s_tile = spool.tile([128, PE_T], dt.float8e4, tag="s")
                    nc.sync.dma_start(
                        s_tile[:], s_d.ap()[jb * 128:(jb + 1) * 128, :])
                    for h in range(H):
                        nc.tensor.matmul(
                            oh[h][:],
                            q_sbuf[:, jb * 128:(jb + 1) * 128],
                            s_tile[:, h * CHUNK:(h + 1) * CHUNK],
                            start=(jb == 0), stop=(jb == JB - 1),
                        )
                # one-hot chunk tails: emitted ahead of the LAST gather
                # chunk's processing so the PE works through them while the
                # Pool engine is still generating chunk NG-1's descriptors
                # (m is channel-major -- trans2 reads it directly).
                if c == NG - 1:
                    for h in range(H):
                        ct = NG + h
                        m_cm = mpool.tile([128, CHUNK], dt.float16, tag="mcm")
                        nc.scalar.activation(
                            m_cm[:], oh[h][:],
                            mybir.ActivationFunctionType.Copy)
                        osb2 = opool.tile([128, CHUNK // 128 * C_OUT],
                                          dt.float32, tag="osb2")
                        for s in range(CHUNK // 128):
                            ops = po.tile([128, C_OUT], dt.float32)
                            nc.tensor.matmul(
                                ops[:], m_cm[:, s * 128:(s + 1) * 128],
                                w2T[:], start=True, stop=True)
                            nc.vector.tensor_add(
                                osb2[:, s * C_OUT:(s + 1) * C_OUT],
                                ops[:], b2[:])
                        nc.sync.dma_start(
                            out_d.ap()[ct * CHUNK:(ct + 1) * CHUNK, :]
                            .rearrange("(s p) o -> p s o", p=128),
                            osb2[:].rearrange("p (s o) -> p s o", o=C_OUT),
                        )
                # K-sum: k 0..5 on PE (identity-accumulate), k 6..11 as one
                # DVE strided reduce; combine is the PSUM->SBUF add.  The
                # PE carries the one-hot matmuls too, so it gets the
                # smaller half.
                m_f16 = mpool.tile([128, CHUNK], dt.float16)  # token-major
                part = mpool.tile([128, CHUNK], dt.float32, tag="part")
                nc.vector.reduce_sum(
                    part[:],
                    g[:, 6 * (CHUNK // 128):, :]
                    .rearrange("p (k t) c -> p t c k", k=6),
                    axis=mybir.AxisListType.X,
                )
                mps = pp.tile([128, CHUNK], dt.float32, tag="ps512")
                for kb in range(6):
                    nc.tensor.matmul(
                        mps[:],
                        ident[:],
                        g[:, kb * (CHUNK // 128):(kb + 1) * (CHUNK // 128), :],
                        start=(kb == 0), stop=(kb == 5),
                    )
                nc.vector.tensor_add(m_f16[:], mps[:], part[:])

                osb = opool.tile([128, CHUNK // 128 * C_OUT], dt.float32)
                for s in range(CHUNK // 128):
                    tps = pt.tile([128, 128], dt.float16, tag="tps")
                    nc.tensor.transpose(
                        tps[:], m_f16[:, s * 128:(s + 1) * 128], ident[:])
                    mt = mtpool.tile([128, 128], dt.float16)  # [ch, tok]
                    nc.scalar.activation(
                        mt[:], tps[:], mybir.ActivationFunctionType.Copy)
                    ops = po.tile([128, C_OUT], dt.float32)
                    nc.tensor.matmul(ops[:], mt[:], w2T[:],
                                     start=True, stop=True)
                    nc.vector.tensor_add(
                        osb[:, s * C_OUT:(s + 1) * C_OUT], ops[:], b2[:])
                nc.sync.dma_start(
                    out_d.ap()[c * CHUNK:(c + 1) * CHUNK, :]
                    .rearrange("(s p) o -> p s o", p=128),
                    osb[:].rearrange("p (s o) -> p s o", o=C_OUT),
                )

    nc.compile()
    return nc


def _get_program():
    if "nc" not in _CACHE:
        _CACHE["nc"] = _build_program()
    return _CACHE["nc"]


def _host_prep(x, knn_idx, W1, b1, Wc0, bc0, Wc1, bc1, W2, b2):
    """Fuse weights and build per-core input maps."""
    f64 = np.float64
    weff = (Wc0.astype(f64) @ W1.astype(f64))                    # [128, 3]
    beff = (Wc0.astype(f64) @ b1.astype(f64) + bc0.astype(f64))  # [128]
    w2s = W2.astype(f64) / K                                     # fold 1/K

    weffT = np.ascontiguousarray(weff.T.astype(np.float16))
    beff_c = np.ascontiguousarray(beff.astype(np.float32)[:, None])
    wc1T = np.ascontiguousarray(Wc1.T.astype(np.float16))
    bc1_c = np.ascontiguousarray(bc1.astype(np.float32)[:, None])
    w2T = np.ascontiguousarray(w2s.T.astype(np.float16))
    b2_b = np.ascontiguousarray(np.tile(b2.astype(np.float32)[None, :], (128, 1)))
    ident = np.eye(128, dtype=np.float16)

    tcol = np.repeat(np.arange(PE_T), K)
    in_maps = []
    for bi in range(B):
        xT = np.ascontiguousarray(x[bi].T.astype(np.float16))
        kb = knn_idx[bi].astype(np.int16)
        # gather-chunk idx: k-major flat list wrapped into 16 partitions,
        # replicated to all 8 Q7 core groups (128 partitions).
        cols = []
        for c in range(NG):
            flat = np.ascontiguousarray(
                kb[c * CHUNK:(c + 1) * CHUNK, :].T).reshape(-1)  # k-major
            wrapped = flat.reshape(IDX_SLOTS, 16).T
            cols.append(np.tile(wrapped, (8, 1)))
        idxw = np.ascontiguousarray(np.concatenate(cols, axis=1))
        # one-hot selection matrix for the tail tokens
        sel32 = np.zeros((N, PE_T), np.int32)
        np.add.at(sel32,
                  (knn_idx[bi, NG * CHUNK:, :].astype(np.int64).ravel(), tcol),
                  1)
        sel = sel32.astype(ml_dtypes.float8_e4m3)
        in_maps.append({
            "xT": xT, "idx": idxw, "weffT": weffT, "beff": beff_c,
            "wc1T": wc1T, "bc1c": bc1_c, "w2T": w2T, "b2b": b2_b,
            "ident": ident, "sel": sel,
        })
    return in_maps


def kernel(x, knn_idx, W1, b1, Wc0, bc0, Wc1, bc1, W2, b2):
    x = np.asarray(x)
    knn_idx = np.asarray(knn_idx)
    args = [np.asarray(a) for a in (W1, b1, Wc0, bc0, Wc1, bc1, W2, b2)]
    in_maps = _host_prep(x, knn_idx, *args)
    nc = _get_program()
    from concourse import bass_utils
    res = bass_utils.run_bass_kernel_spmd(nc, in_maps, core_ids=list(range(B)))
    return np.stack([res.results[i]["out"] for i in range(B)], axis=0)


# revision 26
# speedup vs baseline: 1.0806x; 1.0806x over previous
"""Trainium2 Bass kernel for nn_AugmentShallow (gnn_message_passing).

Strength-reduced formulation (see kernel.py docstring): per-point MLP
    p[j] = relu(Weff @ x[j] + beff),  q[j] = relu(Wc1 @ p[j] + bc1)
then m[n] = sum_k q[knn[n,k]] and out = m @ (W2/K).T + b2.

The neighbor-sum is split across two engines that run concurrently:
  * chunks 0..11 (tokens 0..6143): SWDGE dma_gather of q rows from DRAM
    (non-transpose, 4 queues) + PE identity-matmul / DVE strided-reduce
    K-sum.  Q7 descriptor generation (~2.35ns/idx, serial) is the
    critical path of this half.
  * chunks 12..15 (tokens 6144..8191): PE one-hot matmul
    m_cm = sum_jb q_jb.T @ S[jb]  where S is the host-built selection
    matrix ([8192, 2048] fp8, S[j,t] = multiplicity of j among token
    t's neighbors).  The 64 jb-block matmuls are interleaved with the
    gather-chunk emission so they fill the PE while the Pool engine
    generates descriptors.  Output is channel-major, so trans2 consumes
    it directly (no transposes on this half).

Sharding: data-parallel over B -- core i owns batch i.
"""

import sys

if "/opt/trn_rl_repo" not in sys.path:
    sys.path.insert(0, "/opt/trn_rl_repo")

import numpy as np
import ml_dtypes

B, N, K = 8, 8192, 12
C_IN, C_HID, C_OUT = 3, 128, 256

CHUNK = 512                     # output tokens per gather chunk
N_CHUNKS = N // CHUNK           # 16
H = 3                           # chunks handled by the PE one-hot path
NG = N_CHUNKS - H               # gather chunks
PE_T = H * CHUNK                # one-hot tokens (tail of the batch)
IDX_PER_CHUNK = CHUNK * K       # 6144
IDX_SLOTS = IDX_PER_CHUNK // 16  # 384 (16-partition wrap)
G_SLOTS = IDX_PER_CHUNK // 128  # 48
P_TOK = 512                     # tokens per p/q-stage matmul
N_QUEUES = 4
JB = N // 128                   # 64 one-hot contraction blocks

_CACHE = {}


def _build_program():
    import concourse.bacc as bacc
    import concourse.mybir as mybir
    import concourse.tile as tile

    dt = mybir.dt
    nc = bacc.Bacc("TRN2", target_bir_lowering=False, debug=False, num_devices=8,
                   num_swdge_queues=N_QUEUES)

    xT_d = nc.dram_tensor("xT", [C_IN, N], dt.float16, kind="ExternalInput")
    idx_d = nc.dram_tensor("idx", [128, NG * IDX_SLOTS], dt.int16,
                           kind="ExternalInput")
    weffT_d = nc.dram_tensor("weffT", [C_IN, C_HID], dt.float16,
                             kind="ExternalInput")
    beff_d = nc.dram_tensor("beff", [C_HID, 1], dt.float32, kind="ExternalInput")
    wc1T_d = nc.dram_tensor("wc1T", [C_HID, C_HID], dt.float16,
                            kind="ExternalInput")
    bc1_d = nc.dram_tensor("bc1c", [C_HID, 1], dt.float32, kind="ExternalInput")
    w2T_d = nc.dram_tensor("w2T", [C_HID, C_OUT], dt.float16,
                           kind="ExternalInput")
    b2_d = nc.dram_tensor("b2b", [128, C_OUT], dt.float32, kind="ExternalInput")
    ident_d = nc.dram_tensor("ident", [128, 128], dt.float16,
                             kind="ExternalInput")
    s_d = nc.dram_tensor("sel", [N, PE_T], dt.float8e4, kind="ExternalInput")
    out_d = nc.dram_tensor("out", [N, C_OUT], dt.float32, kind="ExternalOutput")

    with tile.TileContext(nc) as tc:
        with (
            tc.tile_pool(name="const", bufs=1) as cpool,
            tc.tile_pool(name="qstage", bufs=2) as qpool,
            tc.tile_pool(name="spool", bufs=3) as spool,
            tc.tile_pool(name="gpool", bufs=9) as gpool,
            tc.tile_pool(name="mpool", bufs=4) as mpool,
            tc.tile_pool(name="mtpool", bufs=3) as mtpool,
            tc.tile_pool(name="opool", bufs=2) as opool,
            tc.tile_pool(name="qdram", bufs=1, space="DRAM") as dpool,
            tc.tile_pool(name="pp", bufs=2, space="PSUM") as pp,
            tc.tile_pool(name="oh", bufs=1, space="PSUM") as ohpool,
            tc.tile_pool(name="pt", bufs=2, space="PSUM") as pt,
            tc.tile_pool(name="po", bufs=1, space="PSUM") as po,
        ):
            # ---- persistent SBUF tensors -------------------------------
            xT = cpool.tile([C_IN, N], dt.float16)
            idx = cpool.tile([128, NG * IDX_SLOTS], dt.int16)
            weffT = cpool.tile([C_IN, C_HID], dt.float16)
            beff = cpool.tile([C_HID, 1], dt.float32)
            wc1T = cpool.tile([C_HID, C_HID], dt.float16)
            bc1 = cpool.tile([C_HID, 1], dt.float32)
            w2T = cpool.tile([C_HID, C_OUT], dt.float16)
            b2 = cpool.tile([128, C_OUT], dt.float32)
            ident = cpool.tile([128, 128], dt.float16)
            p_f16 = cpool.tile([128, N], dt.float16)   # [ch, tok]
            q_sbuf = cpool.tile([128, N], dt.float16)  # token-major stripes
            q_dram = dpool.tile([N, C_HID], dt.float16)  # token-major rows

            nc.sync.dma_start(idx[:], idx_d.ap()[:])
            nc.sync.dma_start(xT[:], xT_d.ap()[:])
            nc.sync.dma_start(weffT[:], weffT_d.ap()[:])
            nc.sync.dma_start(beff[:], beff_d.ap()[:])
            nc.sync.dma_start(wc1T[:], wc1T_d.ap()[:])
            nc.sync.dma_start(bc1[:], bc1_d.ap()[:])
            nc.sync.dma_start(w2T[:], w2T_d.ap()[:])
            nc.sync.dma_start(b2[:], b2_d.ap()[:])
            nc.sync.dma_start(ident[:], ident_d.ap()[:])

            # ---- p = relu(Weff @ x + beff), channel-major [128, N] -----
            for c in range(N // P_TOK):
                ppt = pp.tile([128, P_TOK], dt.float32, tag="ps512")
                nc.tensor.matmul(
                    ppt[:], weffT[:], xT[:, c * P_TOK:(c + 1) * P_TOK],
                    start=True, stop=True,
                )
                nc.scalar.activation(
                    p_f16[:, c * P_TOK:(c + 1) * P_TOK], ppt[:],
                    mybir.ActivationFunctionType.Relu, bias=beff[:],
                )

            # ---- q = relu(Wc1 @ p + bc1); PE-transpose to token-major
            # stripes in q_sbuf (token j at partition j%128, stripe j//128),
            # which both the DRAM staging DMA and the one-hot lhsT use.
            for g4 in range(N // P_TOK):
                qps = pp.tile([128, P_TOK], dt.float32, tag="ps512")
                nc.tensor.matmul(
                    qps[:], wc1T[:], p_f16[:, g4 * P_TOK:(g4 + 1) * P_TOK],
                    start=True, stop=True,
                )
                q_cm = qpool.tile([128, P_TOK], dt.float16, tag="qcm")
                nc.scalar.activation(
                    q_cm[:], qps[:],
                    mybir.ActivationFunctionType.Relu, bias=bc1[:],
                )
                for s in range(P_TOK // 128):
                    tb = g4 * (P_TOK // 128) + s
                    tq = pt.tile([128, 128], dt.float16, tag="tps")
                    nc.tensor.transpose(
                        tq[:], q_cm[:, s * 128:(s + 1) * 128], ident[:])
                    # DVE copy: the head's ACT chain (p/q relus) is the
                    # longer pole; DVE is idle here.
                    nc.vector.tensor_copy(
                        q_sbuf[:, tb * 128:(tb + 1) * 128], tq[:])
                nc.sync.dma_start(
                    q_dram[g4 * P_TOK:(g4 + 1) * P_TOK, :]
                    .rearrange("(s p) o -> p s o", p=128),
                    q_sbuf[:, g4 * P_TOK:(g4 + 1) * P_TOK]
                    .rearrange("p (s o) -> p s o", o=C_HID),
                )

            # one-hot accumulators, live through the whole gather phase
            oh = [ohpool.tile([128, CHUNK], dt.float32, tag=f"oh{h}",
                              name=f"oh{h}")
                  for h in range(H)]
            # spread the 64 jb one-hot steps across gather iterations
            # 1..NG-2: iteration 0 keeps the head's DMA bandwidth for the
            # q_dram staging, and finishing early lets the one-hot chunk
            # tails overlap the last gather chunk.
            NSL = NG - 2
            jb_sched = [[] for _ in range(NG)]
            for i in range(NSL):
                jb_sched[i + 1] = list(range((i * JB) // NSL,
                                             ((i + 1) * JB) // NSL))

            # ---- gather chunks + interleaved one-hot -------------------
            for c in range(NG):
                g = gpool.tile([128, G_SLOTS, 128], dt.float16)
                nc.gpsimd.dma_gather(
                    g[:],
                    q_dram[:],
                    idx[:, c * IDX_SLOTS:(c + 1) * IDX_SLOTS],
                    num_idxs=IDX_PER_CHUNK,
                    num_idxs_reg=IDX_PER_CHUNK,
                    elem_size=C_HID,
                    transpose=False,
                    single_packet=False,
                    queue_num=c % N_QUEUES,
                )
                # one-hot slice: fills the PE while Q7 generates descs
                for jb in jb_sched[c]:
                    s_tile = spool.tile([128, PE_T], dt.float8e4, tag="s")
                    nc.sync.dma_start(
                        s_tile[:], s_d.ap()[jb * 128:(jb + 1) * 128, :])
                    for h in range(H):
                        nc.tensor.matmul(
                            oh[h][:],
                            q_sbuf[:, jb * 128:(jb + 1) * 128],
                            s_tile[:, h * CHUNK:(h + 1) * CHUNK],
                            start=(jb == 0), stop=(jb == JB - 1),
                        )
                # one-hot chunk tails: emitted ahead of the LAST gather
                # chunk's processing so the PE works through them while the
                # Pool engine is still generating chunk NG-1's descriptors
                # (m is channel-major -- trans2 reads it directly).
                if c == NG - 1:
                    for h in range(H):
                        ct = NG + h
                        m_cm = mpool.tile([128, CHUNK], dt.float16, tag="mcm")
                        nc.scalar.activation(
                            m_cm[:], oh[h][:],
                            mybir.ActivationFunctionType.Copy)
                        osb2 = opool.tile([128, CHUNK // 128 * C_OUT],
                                          dt.float32, tag="osb2")
                        for s in range(CHUNK // 128):
                            ops = po.tile([128, C_OUT], dt.float32)
                            nc.tensor.matmul(
                                ops[:], m_cm[:, s * 128:(s + 1) * 128],
                                w2T[:], start=True, stop=True)
                            nc.vector.tensor_add(
                                osb2[:, s * C_OUT:(s + 1) * C_OUT],
                                ops[:], b2[:])
                        nc.sync.dma_start(
                            out_d.ap()[ct * CHUNK:(ct + 1) * CHUNK, :]
                            .rearrange("(s p) o -> p s o", p=128),
                            osb2[:].rearrange("p (s o) -> p s o", o=C_OUT),
                        )
                # K-sum: k 0..7 on PE (identity-accumulate), k 8..11 as one
                # DVE strided reduce; combine is the PSUM->SBUF add.
                m_f16 = mpool.tile([128, CHUNK], dt.float16)  # token-major
                part = mpool.tile([128, CHUNK], dt.float32, tag="part")
                nc.vector.reduce_sum(
                    part[:],
                    g[:, 8 * (CHUNK // 128):, :]
                    .rearrange("p (k t) c -> p t c k", k=4),
                    axis=mybir.AxisListType.X,
                )
                mps = pp.tile([128, CHUNK], dt.float32, tag="ps512")
                for kb in range(8):
                    nc.tensor.matmul(
                        mps[:],
                        ident[:],
                        g[:, kb * (CHUNK // 128):(kb + 1) * (CHUNK // 128), :],
                        start=(kb == 0), stop=(kb == 7),
                    )
                nc.vector.tensor_add(m_f16[:], mps[:], part[:])

                osb = opool.tile([128, CHUNK // 128 * C_OUT], dt.float32)
                for s in range(CHUNK // 128):
                    tps = pt.tile([128, 128], dt.float16, tag="tps")
                    nc.tensor.transpose(
                        tps[:], m_f16[:, s * 128:(s + 1) * 128], ident[:])
                    mt = mtpool.tile([128, 128], dt.float16)  # [ch, tok]
                    nc.scalar.activation(
                        mt[:], tps[:], mybir.ActivationFunctionType.Copy)
                    ops = po.tile([128, C_OUT], dt.float32)
                    nc.tensor.matmul(ops[:], mt[:], w2T[:],
                                     start=True, stop=True)
                    nc.vector.tensor_add(
                        osb[:, s * C_OUT:(s + 1) * C_OUT], ops[:], b2[:])
                nc.sync.dma_start(
                    out_d.ap()[c * CHUNK:(c + 1) * CHUNK, :]
                    .rearrange("(s p) o -> p s o", p=128),
                    osb[:].rearrange("p (s o) -> p s o", o=C_OUT),
                )

    nc.compile()
    return nc


def _get_program():
    if "nc" not in _CACHE:
        _CACHE["nc"] = _build_program()
    return _CACHE["nc"]


def _host_prep(x, knn_idx, W1, b1, Wc0, bc0, Wc1, bc1, W2, b2):
    """Fuse weights and build per-core input maps."""
    f64 = np.float64
    weff = (Wc0.astype(f64) @ W1.astype(f64))                    # [128, 3]
    beff = (Wc0.astype(f64) @ b1.astype(f64) + bc0.astype(f64))  # [128]
    w2s = W2.astype(f64) / K                                     # fold 1/K

    weffT = np.ascontiguousarray(weff.T.astype(np.float16))
    beff_c = np.ascontiguousarray(beff.astype(np.float32)[:, None])
    wc1T = np.ascontiguousarray(Wc1.T.astype(np.float16))
    bc1_c = np.ascontiguousarray(bc1.astype(np.float32)[:, None])
    w2T = np.ascontiguousarray(w2s.T.astype(np.float16))
    b2_b = np.ascontiguousarray(np.tile(b2.astype(np.float32)[None, :], (128, 1)))
    ident = np.eye(128, dtype=np.float16)

    tcol = np.repeat(np.arange(PE_T), K)
    in_maps = []
    for bi in range(B):
        xT = np.ascontiguousarray(x[bi].T.astype(np.float16))
        kb = knn_idx[bi].astype(np.int16)
        # gather-chunk idx: k-major flat list wrapped into 16 partitions,
        # replicated to all 8 Q7 core groups (128 partitions).
        cols = []
        for c in range(NG):
            flat = np.ascontiguousarray(
                kb[c * CHUNK:(c + 1) * CHUNK, :].T).reshape(-1)  # k-major
            wrapped = flat.reshape(IDX_SLOTS, 16).T
            cols.append(np.tile(wrapped, (8, 1)))
        idxw = np.ascontiguousarray(np.concatenate(cols, axis=1))
        # one-hot selection matrix for the tail tokens
        sel32 = np.zeros((N, PE_T), np.int32)
        np.add.at(sel32,
                  (knn_idx[bi, NG * CHUNK:, :].astype(np.int64).ravel(), tcol),
                  1)
        sel = sel32.astype(ml_dtypes.float8_e4m3)
        in_maps.append({
            "xT": xT, "idx": idxw, "weffT": weffT, "beff": beff_c,
            "wc1T": wc1T, "bc1c": bc1_c, "w2T": w2T, "b2b": b2_b,
            "ident": ident, "sel": sel,
        })
    return in_maps


def kernel(x, knn_idx, W1, b1, Wc0, bc0, Wc1, bc1, W2, b2):
    x = np.asarray(x)
    knn_idx = np.asarray(knn_idx)
    args = [np.asarray(a) for a in (W1, b1, Wc0, bc0, Wc1, bc1, W2, b2)]
    in_maps = _host_prep(x, knn_idx, *args)
    nc = _get_program()
    from concourse import bass_utils
    res = bass_utils.run_bass_kernel_spmd(nc, in_maps, core_ids=list(range(B)))
    return np.stack([res.results[i]["out"] for i in range(B)], axis=0)


# revision 29
# speedup vs baseline: 1.0978x; 1.0159x over previous
"""Trainium2 Bass kernel for nn_AugmentShallow (gnn_message_passing).

Strength-reduced formulation (see kernel.py docstring): per-point MLP
    p[j] = relu(Weff @ x[j] + beff),  q[j] = relu(Wc1 @ p[j] + bc1)
then m[n] = sum_k q[knn[n,k]] and out = m @ (W2/K).T + b2.

The neighbor-sum is split across two engines that run concurrently:
  * chunks 0..11 (tokens 0..6143): SWDGE dma_gather of q rows from DRAM
    (non-transpose, 4 queues) + PE identity-matmul / DVE strided-reduce
    K-sum.  Q7 descriptor generation (~2.35ns/idx, serial) is the
    critical path of this half.
  * chunks 12..15 (tokens 6144..8191): PE one-hot matmul
    m_cm = sum_jb q_jb.T @ S[jb]  where S is the host-built selection
    matrix ([8192, 2048] fp8, S[j,t] = multiplicity of j among token
    t's neighbors).  The 64 jb-block matmuls are interleaved with the
    gather-chunk emission so they fill the PE while the Pool engine
    generates descriptors.  Output is channel-major, so trans2 consumes
    it directly (no transposes on this half).

Sharding: data-parallel over B -- core i owns batch i.
"""

import sys

if "/opt/trn_rl_repo" not in sys.path:
    sys.path.insert(0, "/opt/trn_rl_repo")

import numpy as np
import ml_dtypes

B, N, K = 8, 8192, 12
C_IN, C_HID, C_OUT = 3, 128, 256

CHUNK = 512                     # output tokens per gather chunk
N_CHUNKS = N // CHUNK           # 16
H = 3                           # chunks handled by the PE one-hot path
NG = N_CHUNKS - H               # gather chunks
PE_T = H * CHUNK                # one-hot tokens (tail of the batch)
IDX_PER_CHUNK = CHUNK * K       # 6144
IDX_SLOTS = IDX_PER_CHUNK // 16  # 384 (16-partition wrap)
G_SLOTS = IDX_PER_CHUNK // 128  # 48
P_TOK = 512                     # tokens per p/q-stage matmul
N_QUEUES = 4
JB = N // 128                   # 64 one-hot contraction blocks
GATHER_T = (N_CHUNKS - H) * CHUNK           # 6656 gather-path tokens
GCH = [CHUNK] * 12 + [CHUNK // 2] * 2       # graded gather chunk sizes

_CACHE = {}


def _build_program():
    import concourse.bacc as bacc
    import concourse.mybir as mybir
    import concourse.tile as tile

    dt = mybir.dt
    nc = bacc.Bacc("TRN2", target_bir_lowering=False, debug=False, num_devices=8,
                   num_swdge_queues=N_QUEUES)

    xT_d = nc.dram_tensor("xT", [C_IN, N], dt.float16, kind="ExternalInput")
    idx_d = nc.dram_tensor("idx", [128, NG * IDX_SLOTS], dt.int16,
                           kind="ExternalInput")
    weffT_d = nc.dram_tensor("weffT", [C_IN, C_HID], dt.float16,
                             kind="ExternalInput")
    beff_d = nc.dram_tensor("beff", [C_HID, 1], dt.float32, kind="ExternalInput")
    wc1T_d = nc.dram_tensor("wc1T", [C_HID, C_HID], dt.float16,
                            kind="ExternalInput")
    bc1_d = nc.dram_tensor("bc1c", [C_HID, 1], dt.float32, kind="ExternalInput")
    w2T_d = nc.dram_tensor("w2T", [C_HID, C_OUT], dt.float16,
                           kind="ExternalInput")
    b2_d = nc.dram_tensor("b2b", [128, C_OUT], dt.float32, kind="ExternalInput")
    ident_d = nc.dram_tensor("ident", [128, 128], dt.float16,
                             kind="ExternalInput")
    s_d = nc.dram_tensor("sel", [N, PE_T], dt.float8e4, kind="ExternalInput")
    out_d = nc.dram_tensor("out", [N, C_OUT], dt.float32, kind="ExternalOutput")

    with tile.TileContext(nc) as tc:
        with (
            tc.tile_pool(name="const", bufs=1) as cpool,
            tc.tile_pool(name="qstage", bufs=2) as qpool,
            tc.tile_pool(name="spool", bufs=3) as spool,
            tc.tile_pool(name="gpool", bufs=9) as gpool,
            tc.tile_pool(name="mpool", bufs=4) as mpool,
            tc.tile_pool(name="mtpool", bufs=3) as mtpool,
            tc.tile_pool(name="opool", bufs=2) as opool,
            tc.tile_pool(name="qdram", bufs=1, space="DRAM") as dpool,
            tc.tile_pool(name="pp", bufs=2, space="PSUM") as pp,
            tc.tile_pool(name="oh", bufs=1, space="PSUM") as ohpool,
            tc.tile_pool(name="pt", bufs=2, space="PSUM") as pt,
            tc.tile_pool(name="po", bufs=1, space="PSUM") as po,
        ):
            # ---- persistent SBUF tensors -------------------------------
            xT = cpool.tile([C_IN, N], dt.float16)
            idx = cpool.tile([128, NG * IDX_SLOTS], dt.int16)
            weffT = cpool.tile([C_IN, C_HID], dt.float16)
            beff = cpool.tile([C_HID, 1], dt.float32)
            wc1T = cpool.tile([C_HID, C_HID], dt.float16)
            bc1 = cpool.tile([C_HID, 1], dt.float32)
            w2T = cpool.tile([C_HID, C_OUT], dt.float16)
            b2 = cpool.tile([128, C_OUT], dt.float32)
            ident = cpool.tile([128, 128], dt.float16)
            p_f16 = cpool.tile([128, N], dt.float16)   # [ch, tok]
            q_sbuf = cpool.tile([128, N], dt.float16)  # token-major stripes
            q_dram = dpool.tile([N, C_HID], dt.float16)  # token-major rows

            nc.sync.dma_start(idx[:], idx_d.ap()[:])
            nc.sync.dma_start(xT[:], xT_d.ap()[:])
            nc.sync.dma_start(weffT[:], weffT_d.ap()[:])
            nc.sync.dma_start(beff[:], beff_d.ap()[:])
            nc.sync.dma_start(wc1T[:], wc1T_d.ap()[:])
            nc.sync.dma_start(bc1[:], bc1_d.ap()[:])
            nc.sync.dma_start(w2T[:], w2T_d.ap()[:])
            nc.sync.dma_start(b2[:], b2_d.ap()[:])
            nc.sync.dma_start(ident[:], ident_d.ap()[:])

            # ---- p = relu(Weff @ x + beff), channel-major [128, N] -----
            for c in range(N // P_TOK):
                ppt = pp.tile([128, P_TOK], dt.float32, tag="ps512")
                nc.tensor.matmul(
                    ppt[:], weffT[:], xT[:, c * P_TOK:(c + 1) * P_TOK],
                    start=True, stop=True,
                )
                # relu(x + beff) on DVE: the head's critical chain is the
                # ACT engine (q-stage relus); DVE is idle here.
                nc.vector.tensor_scalar(
                    p_f16[:, c * P_TOK:(c + 1) * P_TOK], ppt[:],
                    scalar1=beff[:], scalar2=0.0,
                    op0=mybir.AluOpType.add, op1=mybir.AluOpType.max,
                )

            # ---- q = relu(Wc1 @ p + bc1); PE-transpose to token-major
            # stripes in q_sbuf (token j at partition j%128, stripe j//128),
            # which both the DRAM staging DMA and the one-hot lhsT use.
            for g4 in range(N // P_TOK):
                qps = pp.tile([128, P_TOK], dt.float32, tag="ps512")
                nc.tensor.matmul(
                    qps[:], wc1T[:], p_f16[:, g4 * P_TOK:(g4 + 1) * P_TOK],
                    start=True, stop=True,
                )
                q_cm = qpool.tile([128, P_TOK], dt.float16, tag="qcm")
                nc.scalar.activation(
                    q_cm[:], qps[:],
                    mybir.ActivationFunctionType.Relu, bias=bc1[:],
                )
                for s in range(P_TOK // 128):
                    tb = g4 * (P_TOK // 128) + s
                    tq = pt.tile([128, 128], dt.float16, tag="tps")
                    nc.tensor.transpose(
                        tq[:], q_cm[:, s * 128:(s + 1) * 128], ident[:])
                    # DVE copy: the head's ACT chain (p/q relus) is the
                    # longer pole; DVE is idle here.
                    nc.vector.tensor_copy(
                        q_sbuf[:, tb * 128:(tb + 1) * 128], tq[:])
                nc.sync.dma_start(
                    q_dram[g4 * P_TOK:(g4 + 1) * P_TOK, :]
                    .rearrange("(s p) o -> p s o", p=128),
                    q_sbuf[:, g4 * P_TOK:(g4 + 1) * P_TOK]
                    .rearrange("p (s o) -> p s o", o=C_HID),
                )

            # one-hot accumulators, live through the whole gather phase
            oh = [ohpool.tile([128, CHUNK], dt.float32, tag=f"oh{h}",
                              name=f"oh{h}")
                  for h in range(H)]
            # spread the 64 jb one-hot steps across gather iterations
            # 1..NG-2: iteration 0 keeps the head's DMA bandwidth for the
            # q_dram staging, and finishing early lets the one-hot chunk
            # tails overlap the last gather chunk.
            NSL = len(GCH) - 3
            jb_sched = [[] for _ in range(len(GCH))]
            for i in range(NSL):
                jb_sched[i + 1] = list(range((i * JB) // NSL,
                                             ((i + 1) * JB) // NSL))

            # ---- gather chunks + interleaved one-hot -------------------
            # graded sizes: the final two chunks are half-size so the last
            # drain + K-sum + trans2 tail is shorter.
            tok0 = 0
            sl0 = 0
            for c, ctok in enumerate(GCH):
                cidx = ctok * K
                csl = cidx // 16
                gsl = cidx // 128
                g = gpool.tile([128, G_SLOTS, 128], dt.float16, tag="g")
                nc.gpsimd.dma_gather(
                    g[:, :gsl, :],
                    q_dram[:],
                    idx[:, sl0:sl0 + csl],
                    num_idxs=cidx,
                    num_idxs_reg=cidx,
                    elem_size=C_HID,
                    transpose=False,
                    single_packet=False,
                    queue_num=c % N_QUEUES,
                )
                # one-hot slice: fills the PE while Q7 generates descs
                for jb in jb_sched[c]:
                    s_tile = spool.tile([128, PE_T], dt.float8e4, tag="s")
                    nc.sync.dma_start(
                        s_tile[:], s_d.ap()[jb * 128:(jb + 1) * 128, :])
                    for h in range(H):
                        nc.tensor.matmul(
                            oh[h][:],
                            q_sbuf[:, jb * 128:(jb + 1) * 128],
                            s_tile[:, h * CHUNK:(h + 1) * CHUNK],
                            start=(jb == 0), stop=(jb == JB - 1),
                        )
                # one-hot chunk tails: emitted ahead of the LAST gather
                # chunk's processing so the PE works through them while the
                # Pool engine is still generating its descriptors (m is
                # channel-major -- trans2 reads it directly).
                if c == len(GCH) - 1:
                    for h in range(H):
                        ct0 = GATHER_T + h * CHUNK
                        m_cm = mpool.tile([128, CHUNK], dt.float16, tag="mcm")
                        nc.scalar.activation(
                            m_cm[:], oh[h][:],
                            mybir.ActivationFunctionType.Copy)
                        osb2 = opool.tile([128, CHUNK // 128 * C_OUT],
                                          dt.float32, tag="osb2")
                        for s in range(CHUNK // 128):
                            ops = po.tile([128, C_OUT], dt.float32)
                            nc.tensor.matmul(
                                ops[:], m_cm[:, s * 128:(s + 1) * 128],
                                w2T[:], start=True, stop=True)
                            nc.vector.tensor_add(
                                osb2[:, s * C_OUT:(s + 1) * C_OUT],
                                ops[:], b2[:])
                        nc.sync.dma_start(
                            out_d.ap()[ct0:ct0 + CHUNK, :]
                            .rearrange("(s p) o -> p s o", p=128),
                            osb2[:].rearrange("p (s o) -> p s o", o=C_OUT),
                        )
                # K-sum: k 0..7 on PE (identity-accumulate), k 8..11 as one
                # DVE strided reduce; combine is the PSUM->SBUF add.
                tb = ctok // 128
                m_f16 = mpool.tile([128, CHUNK], dt.float16)  # token-major
                part = mpool.tile([128, CHUNK], dt.float32, tag="part")
                nc.vector.reduce_sum(
                    part[:, :ctok],
                    g[:, 8 * tb:12 * tb, :]
                    .rearrange("p (k t) c -> p t c k", k=4),
                    axis=mybir.AxisListType.X,
                )
                mps = pp.tile([128, CHUNK], dt.float32, tag="ps512")
                for kb in range(8):
                    nc.tensor.matmul(
                        mps[:, :ctok],
                        ident[:],
                        g[:, kb * tb:(kb + 1) * tb, :],
                        start=(kb == 0), stop=(kb == 7),
                    )
                nc.vector.tensor_add(m_f16[:, :ctok], mps[:, :ctok],
                                     part[:, :ctok])

                osb = opool.tile([128, CHUNK // 128 * C_OUT], dt.float32)
                for s in range(tb):
                    tps = pt.tile([128, 128], dt.float16, tag="tps")
                    nc.tensor.transpose(
                        tps[:], m_f16[:, s * 128:(s + 1) * 128], ident[:])
                    mt = mtpool.tile([128, 128], dt.float16)  # [ch, tok]
                    nc.scalar.activation(
                        mt[:], tps[:], mybir.ActivationFunctionType.Copy)
                    ops = po.tile([128, C_OUT], dt.float32)
                    nc.tensor.matmul(ops[:], mt[:], w2T[:],
                                     start=True, stop=True)
                    nc.vector.tensor_add(
                        osb[:, s * C_OUT:(s + 1) * C_OUT], ops[:], b2[:])
                nc.sync.dma_start(
                    out_d.ap()[tok0:tok0 + ctok, :]
                    .rearrange("(s p) o -> p s o", p=128),
                    osb[:, :tb * C_OUT].rearrange("p (s o) -> p s o", o=C_OUT),
                )
                tok0 += ctok
                sl0 += csl

    nc.compile()
    return nc


def _get_program():
    if "nc" not in _CACHE:
        _CACHE["nc"] = _build_program()
    return _CACHE["nc"]


def _host_prep(x, knn_idx, W1, b1, Wc0, bc0, Wc1, bc1, W2, b2):
    """Fuse weights and build per-core input maps."""
    f64 = np.float64
    weff = (Wc0.astype(f64) @ W1.astype(f64))                    # [128, 3]
    beff = (Wc0.astype(f64) @ b1.astype(f64) + bc0.astype(f64))  # [128]
    w2s = W2.astype(f64) / K                                     # fold 1/K

    weffT = np.ascontiguousarray(weff.T.astype(np.float16))
    beff_c = np.ascontiguousarray(beff.astype(np.float32)[:, None])
    wc1T = np.ascontiguousarray(Wc1.T.astype(np.float16))
    bc1_c = np.ascontiguousarray(bc1.astype(np.float32)[:, None])
    w2T = np.ascontiguousarray(w2s.T.astype(np.float16))
    b2_b = np.ascontiguousarray(np.tile(b2.astype(np.float32)[None, :], (128, 1)))
    ident = np.eye(128, dtype=np.float16)

    tcol = np.repeat(np.arange(PE_T), K)
    in_maps = []
    for bi in range(B):
        xT = np.ascontiguousarray(x[bi].T.astype(np.float16))
        kb = knn_idx[bi].astype(np.int16)
        # gather-chunk idx: k-major flat list wrapped into 16 partitions,
        # replicated to all 8 Q7 core groups (128 partitions).
        cols = []
        t0 = 0
        for ctok in GCH:
            flat = np.ascontiguousarray(
                kb[t0:t0 + ctok, :].T).reshape(-1)  # k-major
            wrapped = flat.reshape(ctok * K // 16, 16).T
            cols.append(np.tile(wrapped, (8, 1)))
            t0 += ctok
        idxw = np.ascontiguousarray(np.concatenate(cols, axis=1))
        # one-hot selection matrix for the tail tokens
        sel32 = np.zeros((N, PE_T), np.int32)
        np.add.at(sel32,
                  (knn_idx[bi, NG * CHUNK:, :].astype(np.int64).ravel(), tcol),
                  1)
        sel = sel32.astype(ml_dtypes.float8_e4m3)
        in_maps.append({
            "xT": xT, "idx": idxw, "weffT": weffT, "beff": beff_c,
            "wc1T": wc1T, "bc1c": bc1_c, "w2T": w2T, "b2b": b2_b,
            "ident": ident, "sel": sel,
        })
    return in_maps


def kernel(x, knn_idx, W1, b1, Wc0, bc0, Wc1, bc1, W2, b2):
    x = np.asarray(x)
    knn_idx = np.asarray(knn_idx)
    args = [np.asarray(a) for a in (W1, b1, Wc0, bc0, Wc1, bc1, W2, b2)]
    in_maps = _host_prep(x, knn_idx, *args)
    nc = _get_program()
    from concourse import bass_utils
    res = bass_utils.run_bass_kernel_spmd(nc, in_maps, core_ids=list(range(B)))
    return np.stack([res.results[i]["out"] for i in range(B)], axis=0)
